# revision 14
# baseline (speedup 1.0000x reference)
"""Bidirectional GRU encoder (T=256, B=64, E=H=512) for 8 Trainium2 NeuronCores.

Sharding: cores 0-3 run the forward direction (batch slices of 16), cores 4-7
run the backward direction on host-reversed token order. Same SPMD program on
all cores; direction lives entirely in the per-core input data.

Per-core layout is fully transposed: h and all gate tensors live as
[128 partitions = H-chunk, batch free] so elementwise ops use all 128 lanes.
The recurrent matmul streams Whh^T fp16 tiles as stationary weights (FWL) with
h as the tiny moving operand; xg(t) = Wih @ emb(t) is precomputed per 32-step
block at N=512 matmul efficiency and injected into PSUM with identity matmuls.
"""

import os
import sys

sys.path.insert(0, "/opt/trn_rl_repo")

import numpy as np

import concourse.bass as bass
import concourse.bacc as bacc
import concourse.mybir as mybir
from concourse.tile import TileContext
from concourse.bass_utils import run_bass_kernel_spmd

T, B, VOCAB, E, H = 256, 64, 32000, 512, 512
NCORES = 8
BL = B // 4          # 16 batch per core, 4 cores per direction
TBLK = 32            # recurrence block (xg precompute + output DMA granularity)
NBLK = T // TBLK
KC = 4               # contraction chunks of 128 (E = H = 512)
M3H = 12             # 3H / 128 output chunks
NIDX = T * BL        # tokens per core

F16 = mybir.dt.float16
F32 = mybir.dt.float32
AF = mybir.ActivationFunctionType

_CACHE = {}

# module-level telemetry for test.py
last_exec_time_ns = None


def _build_nc(use_gather: bool, reps: int = 1):
    nc = bacc.Bacc()
    etab = nc.declare_dram_parameter("etab", [VOCAB, E], F16, isOutput=False)
    idxs = nc.declare_dram_parameter("idxs", [128, NIDX // 16], mybir.dt.int16, isOutput=False)
    if not use_gather:
        embt = nc.declare_dram_parameter("embt", [128, KC, NIDX], F16, isOutput=False)
    wih = nc.declare_dram_parameter("wih_t", [128, KC, 3 * H], F16, isOutput=False)
    whh = nc.declare_dram_parameter("whh_t", [128, KC, 3 * H], F16, isOutput=False)
    xgb = nc.declare_dram_parameter("xgbias", [128, M3H], F32, isOutput=False)
    bhn = nc.declare_dram_parameter("bhhn", [128, KC, BL], F16, isOutput=False)
    idn = nc.declare_dram_parameter("ident", [128, 128], F16, isOutput=False)
    out_h = nc.declare_dram_parameter("out_h", [128, KC, T, BL], F16, isOutput=True)

    with TileContext(nc) as tc:
        with (
            tc.tile_pool(name="const", bufs=1) as cpool,
            tc.tile_pool(name="xg", bufs=2) as xgpool,
            tc.tile_pool(name="hist", bufs=2) as hpool,
            tc.tile_pool(name="g", bufs=3) as gpool,
            tc.tile_pool(name="psg", bufs=2, space="PSUM") as psg,
            tc.tile_pool(name="psx", bufs=2, space="PSUM") as psx,
        ):
            whh_sb = cpool.tile([128, KC, 3 * H], F16, tag="whh")
            nc.sync.dma_start(out=whh_sb, in_=whh[:])
            wih_sb = cpool.tile([128, KC, 3 * H], F16, tag="wih")
            nc.sync.dma_start(out=wih_sb, in_=wih[:])
            xgb_sb = cpool.tile([128, M3H], F32, tag="xgb")
            nc.sync.dma_start(out=xgb_sb, in_=xgb[:])
            bhn_sb = cpool.tile([128, KC, BL], F16, tag="bhn")
            nc.sync.dma_start(out=bhn_sb, in_=bhn[:])
            id_sb = cpool.tile([128, 128], F16, tag="idn")
            nc.sync.dma_start(out=id_sb, in_=idn[:])

            if use_gather:
                from concourse import library_config
                nc.gpsimd.load_library(library_config.mlp)
                idx_sb = cpool.tile([128, NIDX // 16], mybir.dt.int16, tag="idx")
                nc.sync.dma_start(out=idx_sb, in_=idxs[:])
            else:
                emb_sb = cpool.tile([128, KC, NIDX], F16, tag="emb")
                nc.sync.dma_start(out=emb_sb, in_=embt[:])

            z0 = cpool.tile([128, KC, BL], F16, tag="z0")
            nc.gpsimd.memset(z0, 0.0)
            # dependency-free warmup activation: absorbs the ACT table-set
            # load (walrus folds it into the first ACT instruction's waits,
            # which otherwise exceeds the ISA wait-slot limit). Reads its own
            # uninitialized scratch tile so it schedules first.
            warm = cpool.tile([128, 1], F32, tag="warm")
            nc.scalar.activation(warm[:], warm[:], AF.Sigmoid)
            nc.scalar.activation(warm[:], warm[:], AF.Tanh)
            nc.scalar.activation(warm[:], warm[:], AF.Identity)

            for rep in range(reps):
              prev = None
              for blk in range(NBLK):
                # ---- xg for this block: xg^T[m] = Wih^T-chunk.T @ emb^T ----
                NT = TBLK * BL
                if use_gather:
                    emb_blk = xgpool.tile([128, KC, NT], F16, tag="embblk")
                    gsem = nc.alloc_semaphore(f"gsem_{rep}_{blk}")
                    nc.gpsimd.dma_gather(
                        emb_blk[:], etab[:],
                        idx_sb[:, blk * (NT // 16) : (blk + 1) * (NT // 16)],
                        NT, NT, E, transpose=True, single_packet=False,
                        prepare_only=True, sem=gsem)
                    nc.gpsimd.trigger_dma(count=1)
                    nc.gpsimd.wait_ge(gsem, 16)
                    # identity affine_select AFTER the engine-blocking wait:
                    # gives Tile a post-landing write access on emb_blk so PE
                    # consumers order against completed data, not the prep.
                    nc.gpsimd.affine_select(
                        out=emb_blk[:, 0, 0:1], in_=emb_blk[:, 0, 0:1],
                        compare_op=mybir.AluOpType.is_equal, fill=0.0,
                        base=0, pattern=[[0, 1]], channel_multiplier=0)
                else:
                    emb_blk = emb_sb[:, :, blk * NT : (blk + 1) * NT]
                xg_sb = xgpool.tile([128, M3H, TBLK, BL], F16, tag="xg")
                for m in range(M3H):
                    pxg = psx.tile([128, TBLK, BL], F32, tag="pxg")
                    for k in range(KC):
                        nc.tensor.matmul(
                            pxg[:],
                            wih_sb[:, k, 128 * m : 128 * (m + 1)],
                            emb_blk[:, k, :] if use_gather
                            else emb_sb[:, k, blk * NT : (blk + 1) * NT],
                            start=(k == 0),
                            stop=(k == KC - 1),
                        )
                    nc.scalar.activation(
                        xg_sb[:, m, :, :], pxg[:], AF.Identity,
                        bias=xgb_sb[:, m : m + 1],
                    )

                hist = hpool.tile([128, KC, TBLK, BL], F16, tag="hist")
                for tl in range(TBLK):
                    if blk == 0 and tl == 0:
                        hp = z0[:]
                    elif tl == 0:
                        hp = prev[:, :, TBLK - 1, :]
                    else:
                        hp = hist[:, :, tl - 1, :]

                    ps_r = psg.tile([128, KC, BL], F32, tag="ps_r")
                    ps_z = psg.tile([128, KC, BL], F32, tag="ps_z")
                    ps_n = psg.tile([128, KC, BL], F32, tag="ps_n")

                    # inject xg_t (r, z) and bhh_n (n) via identity matmuls
                    nc.tensor.matmul(ps_r[:], id_sb[:], xg_sb[:, 0:4, tl, :],
                                     start=True, stop=False, skip_group_check=True)
                    nc.tensor.matmul(ps_n[:], id_sb[:], bhn_sb[:],
                                     start=True, stop=False, skip_group_check=True)
                    nc.tensor.matmul(ps_z[:], id_sb[:], xg_sb[:, 4:8, tl, :],
                                     start=True, stop=False, skip_group_check=True)

                    # recurrent matmuls; region order r, n, z hides the
                    # sigmoid/tanh chain under the z-region matmuls
                    for mbase, ps in ((0, ps_r), (8, ps_n), (4, ps_z)):
                        for mi in range(4):
                            mm = mbase + mi
                            for k in range(KC):
                                nc.tensor.matmul(
                                    ps[:, mi, :],
                                    whh_sb[:, k, 128 * mm : 128 * (mm + 1)],
                                    hp[:, k, :],
                                    start=False,
                                    stop=(k == KC - 1),
                                    skip_group_check=True,
                                )

                    r_sb = gpool.tile([128, KC, BL], F32, tag="r")
                    nc.scalar.activation(r_sb[:], ps_r[:], AF.Sigmoid)
                    rhn = gpool.tile([128, KC, BL], F32, tag="rhn")
                    nc.vector.tensor_mul(rhn[:], r_sb[:], ps_n[:])
                    pren = gpool.tile([128, KC, BL], F32, tag="pren")
                    nc.vector.tensor_add(pren[:], rhn[:], xg_sb[:, 8:12, tl, :])
                    n_sb = gpool.tile([128, KC, BL], F32, tag="n")
                    nc.scalar.activation(n_sb[:], pren[:], AF.Tanh)
                    z_sb = gpool.tile([128, KC, BL], F32, tag="z")
                    nc.scalar.activation(z_sb[:], ps_z[:], AF.Sigmoid)
                    d_sb = gpool.tile([128, KC, BL], F32, tag="d")
                    nc.vector.tensor_sub(d_sb[:], hp, n_sb[:])
                    zd = gpool.tile([128, KC, BL], F32, tag="zd")
                    nc.vector.tensor_mul(zd[:], z_sb[:], d_sb[:])
                    nc.vector.tensor_add(hist[:, :, tl, :], n_sb[:], zd[:])

                nc.sync.dma_start(
                    out=out_h[:, :, blk * TBLK : (blk + 1) * TBLK, :], in_=hist[:]
                )
                prev = hist
    nc.compile()
    return nc


def _get_nc(use_gather: bool, reps: int = 1):
    key = ("nc", use_gather, reps)
    if key not in _CACHE:
        _CACHE[key] = _build_nc(use_gather, reps)
    return _CACHE[key]


def _prep_dir(Wih, Whh, bih, bhh):
    # lhsT tiles: [p, k, g] = W^T[k*128+p, g] = W[g, k*128+p]
    wih_t = np.ascontiguousarray(
        Wih.T.reshape(KC, 128, 3 * H).transpose(1, 0, 2)
    ).astype(np.float16)
    whh_t = np.ascontiguousarray(
        Whh.T.reshape(KC, 128, 3 * H).transpose(1, 0, 2)
    ).astype(np.float16)
    bias = (bih + bhh).astype(np.float32).copy()
    bias[2 * H :] = bih[2 * H :]  # n chunk: bih only (bhh_n enters before r-mult)
    xgbias = np.ascontiguousarray(bias.reshape(M3H, 128).T).astype(np.float32)
    bhhn = bhh[2 * H :].reshape(KC, 128).T  # [p, c]
    bhhn_bc = np.ascontiguousarray(
        np.broadcast_to(bhhn[:, :, None], (128, KC, BL))
    ).astype(np.float16)
    return wih_t, whh_t, xgbias, bhhn_bc


def kernel(src, len_src, embed_w, Wih_f, Whh_f, bih_f, bhh_f,
           Wih_b, Whh_b, bih_b, bhh_b):
    global last_exec_time_ns
    src = np.asarray(src)
    len_src = np.asarray(len_src)
    embed_w = np.asarray(embed_w, dtype=np.float32)

    # per-sample reversal of the first len tokens (index prep, host-side)
    t = np.arange(T)[None, :]
    L = len_src[:, None].astype(np.int64)
    idx = np.where(t < L, L - 1 - t, t)  # [B, T]
    src_rev = np.take_along_axis(src.T, idx, axis=1).T  # [T, B]

    etab16 = embed_w.astype(np.float16)
    fwd = _prep_dir(np.asarray(Wih_f), np.asarray(Whh_f),
                    np.asarray(bih_f), np.asarray(bhh_f))
    bwd = _prep_dir(np.asarray(Wih_b), np.asarray(Whh_b),
                    np.asarray(bih_b), np.asarray(bhh_b))
    ident = np.eye(128, dtype=np.float16)

    use_gather = os.environ.get("KERNEL_NO_GATHER", "0") != "1"
    trace = os.environ.get("KERNEL_TRACE", "0") == "1"

    in_maps = []
    for c in range(NCORES):
        d = 0 if c < 4 else 1
        b0 = (c % 4) * BL
        s = (src if d == 0 else src_rev)[:, b0 : b0 + BL]  # [T, BL]
        toks = np.ascontiguousarray(s).reshape(-1).astype(np.int16)  # i = t*BL + b
        W = fwd if d == 0 else bwd
        m = {
            "etab": etab16,
            "wih_t": W[0], "whh_t": W[1], "xgbias": W[2], "bhhn": W[3],
            "ident": ident,
        }
        ii = np.arange(NIDX)
        idxs16 = np.zeros((16, NIDX // 16), np.int16)
        idxs16[ii % 16, ii // 16] = toks
        m["idxs"] = np.tile(idxs16, (8, 1))  # replicated across the 8 Q7 cores
        if not use_gather:
            # host fallback: emb^T chunks [p, c, i] = emb[tok_i][c*128+p]
            emb = etab16[toks]  # [NIDX, E]
            m["embt"] = np.ascontiguousarray(
                emb.reshape(NIDX, KC, 128).transpose(2, 1, 0)
            )
        in_maps.append(m)

    reps = int(os.environ.get("KERNEL_REPS", "1"))
    nc = _get_nc(use_gather, reps)
    res = run_bass_kernel_spmd(nc, in_maps, list(range(NCORES)), trace=trace)
    last_exec_time_ns = res.exec_time_ns

    outputs = np.empty((T, B, 2 * H), np.float32)
    for c in range(NCORES):
        d = 0 if c < 4 else 1
        b0 = (c % 4) * BL
        oh = res.results[c]["out_h"]  # [128, KC, T, BL] f16
        h = oh.transpose(2, 3, 1, 0).reshape(T, BL, H).astype(np.float32)
        outputs[:, b0 : b0 + BL, d * H : (d + 1) * H] = h

    hidden = outputs[len_src - 1, np.arange(B), H : 2 * H][None]  # [1, B, H]
    return outputs, hidden


# revision 26
# speedup vs baseline: 2768.2674x; 2768.2674x over previous
"""Bidirectional GRU encoder (T=256, B=64, E=H=512) for 8 Trainium2 NeuronCores.

Sharding (NDIR=2, default): every core runs BOTH directions over a batch slice
of 8 (core c: forward batch [8c, 8c+8) and backward over host-reversed token
order, same slice). The two recurrences are independent, so each direction's
serial gate chain (sigmoid/tanh on ACT, elementwise on DVE) overlaps the other
direction's matmuls — no engine sits idle waiting on the single serial chain.
NDIR=1 fallback: cores 0-3 forward (batch 16), cores 4-7 backward.

Per-core layout is fully transposed: h and all gate tensors live as
[128 partitions = H-chunk, batch free] so elementwise ops use all 128 lanes.
The recurrent matmul streams Whh^T fp16 tiles as stationary weights (FWL) with
h as the tiny moving operand; xg(t) = Wih @ emb(t) is precomputed per 32-step
block at N=256/512 matmul efficiency and injected into PSUM with identity
matmuls. Embeddings are gathered+transposed on-device per block with
dma_gather(transpose=True) from an fp16 copy of the table.
"""

import os
import sys

sys.path.insert(0, "/opt/trn_rl_repo")

import numpy as np

import concourse.bacc as bacc
import concourse.mybir as mybir
from concourse.tile import TileContext
from concourse.bass_utils import run_bass_kernel_spmd

T, B, VOCAB, E, H = 256, 64, 32000, 512, 512
NCORES = 8
TBLK = 32            # recurrence block (xg precompute + output DMA granularity)
NBLK = T // TBLK
KC = 4               # contraction chunks of 128 (E = H = 512)
M3H = 12             # 3H / 128 output chunks

F16 = mybir.dt.float16
F32 = mybir.dt.float32
AF = mybir.ActivationFunctionType

_CACHE = {}

# module-level telemetry for test.py
last_exec_time_ns = None


def _build_nc(use_gather: bool, ndir: int, reps: int = 1):
    BL = 2 * B // (NCORES * ndir)   # batch per core per direction (16 or 8)
    NIDX = T * BL                # tokens per core per direction
    NT = TBLK * BL               # tokens per block per direction

    nc = bacc.Bacc()
    etab = nc.declare_dram_parameter("etab", [VOCAB, E], F16, isOutput=False)
    idxs = nc.declare_dram_parameter("idxs", [ndir, 128, NIDX // 16],
                                     mybir.dt.int16, isOutput=False)
    if not use_gather:
        embt = nc.declare_dram_parameter("embt", [ndir, 128, KC, NIDX], F16,
                                         isOutput=False)
    wih = nc.declare_dram_parameter("wih_t", [ndir, 128, KC, 3 * H], F16,
                                    isOutput=False)
    whh = nc.declare_dram_parameter("whh_t", [ndir, 128, KC, 3 * H], F16,
                                    isOutput=False)
    xgb = nc.declare_dram_parameter("xgbias", [ndir, 128, M3H], F32,
                                    isOutput=False)
    bhn = nc.declare_dram_parameter("bhhn", [ndir, 128, KC, BL], F16,
                                    isOutput=False)
    idn = nc.declare_dram_parameter("ident", [128, 128], F16, isOutput=False)
    out_h = nc.declare_dram_parameter("out_h", [ndir, 128, KC, T, BL], F16,
                                      isOutput=True)

    D = range(ndir)
    with TileContext(nc) as tc:
        with (
            tc.tile_pool(name="const", bufs=1) as cpool,
            tc.tile_pool(name="xg", bufs=2) as xgpool,
            tc.tile_pool(name="hist", bufs=2) as hpool,
            tc.tile_pool(name="g", bufs=3) as gpool,
            tc.tile_pool(name="psg", bufs=2, space="PSUM") as psg,
            tc.tile_pool(name="psx", bufs=2, space="PSUM") as psx,
        ):
            whh_sb, wih_sb, xgb_sb, bhn_sb, idx_sb, emb_sb = [], [], [], [], [], []
            for d in D:
                w1 = cpool.tile([128, KC, 3 * H], F16, tag=f"whh{d}", name=f"whh{d}")
                nc.sync.dma_start(out=w1, in_=whh[d])
                whh_sb.append(w1)
                w2 = cpool.tile([128, KC, 3 * H], F16, tag=f"wih{d}", name=f"wih{d}")
                nc.sync.dma_start(out=w2, in_=wih[d])
                wih_sb.append(w2)
                b1 = cpool.tile([128, M3H], F32, tag=f"xgb{d}", name=f"xgb{d}")
                nc.sync.dma_start(out=b1, in_=xgb[d])
                xgb_sb.append(b1)
                b2 = cpool.tile([128, KC, BL], F16, tag=f"bhn{d}", name=f"bhn{d}")
                nc.sync.dma_start(out=b2, in_=bhn[d])
                bhn_sb.append(b2)
                if use_gather:
                    ix = cpool.tile([128, NIDX // 16], mybir.dt.int16,
                                    tag=f"idx{d}", name=f"idx{d}")
                    nc.sync.dma_start(out=ix, in_=idxs[d])
                    idx_sb.append(ix)
                else:
                    es = cpool.tile([128, KC, NIDX], F16, tag=f"emb{d}", name=f"emb{d}")
                    nc.sync.dma_start(out=es, in_=embt[d])
                    emb_sb.append(es)
            id_sb = cpool.tile([128, 128], F16, tag="idn", name="idn")
            nc.sync.dma_start(out=id_sb, in_=idn[:])
            if use_gather:
                from concourse import library_config
                nc.gpsimd.load_library(library_config.mlp)

            z0 = cpool.tile([128, KC, BL], F16, tag="z0", name="z0")
            nc.gpsimd.memset(z0, 0.0)
            # dependency-free warmup activation: absorbs the ACT table-set
            # load (walrus folds it into the first ACT instruction's waits,
            # which otherwise exceeds the ISA wait-slot limit). Reads its own
            # uninitialized scratch tile so it schedules first.
            warm = cpool.tile([128, 1], F32, tag="warm", name="warm")
            nc.scalar.activation(warm[:], warm[:], AF.Sigmoid)
            nc.scalar.activation(warm[:], warm[:], AF.Tanh)
            nc.scalar.activation(warm[:], warm[:], AF.Identity)

            for rep in range(reps):
              prev = [None] * ndir
              for blk in range(NBLK):
                # ---- xg for this block: xg^T[m] = Wih^T-chunk.T @ emb^T ----
                emb_blk, xg_sb, hist = [], [], []
                for d in D:
                    if use_gather:
                        eb = xgpool.tile([128, KC, NT], F16, tag=f"embblk{d}", name=f"embblk{d}")
                        gsem = nc.alloc_semaphore(f"gsem_{rep}_{blk}_{d}")
                        nc.gpsimd.dma_gather(
                            eb[:], etab[:],
                            idx_sb[d][:, blk * (NT // 16):(blk + 1) * (NT // 16)],
                            NT, NT, E, transpose=True, single_packet=False,
                            prepare_only=True, sem=gsem)
                        nc.gpsimd.trigger_dma(count=1)
                        nc.gpsimd.wait_ge(gsem, 16)
                        # identity affine_select AFTER the engine-blocking
                        # wait: gives Tile a post-landing write access on the
                        # tile so PE consumers order against completed data.
                        nc.gpsimd.affine_select(
                            out=eb[:, 0, 0:1], in_=eb[:, 0, 0:1],
                            compare_op=mybir.AluOpType.is_equal, fill=0.0,
                            base=0, pattern=[[0, 1]], channel_multiplier=0)
                        emb_blk.append(eb)
                    else:
                        emb_blk.append(emb_sb[d][:, :, blk * NT:(blk + 1) * NT])
                    xs = xgpool.tile([128, M3H, TBLK, BL], F16, tag=f"xg{d}", name=f"xg{d}")
                    xg_sb.append(xs)
                    for m in range(M3H):
                        pxg = psx.tile([128, TBLK, BL], F32, tag="pxg", name="pxg")
                        for k in range(KC):
                            nc.tensor.matmul(
                                pxg[:],
                                wih_sb[d][:, k, 128 * m:128 * (m + 1)],
                                emb_blk[d][:, k, :],
                                start=(k == 0),
                                stop=(k == KC - 1),
                            )
                        nc.scalar.activation(
                            xs[:, m, :, :], pxg[:], AF.Identity,
                            bias=xgb_sb[d][:, m:m + 1],
                        )
                    hist.append(hpool.tile([128, KC, TBLK, BL], F16,
                                           tag=f"hist{d}", name=f"hist{d}"))

                for tl in range(TBLK):
                    hp, psr, psz, psn = [], [], [], []
                    for d in D:
                        if blk == 0 and tl == 0:
                            hp.append(z0[:])
                        elif tl == 0:
                            hp.append(prev[d][:, :, TBLK - 1, :])
                        else:
                            hp.append(hist[d][:, :, tl - 1, :])
                        if ndir == 2:
                            ps_all = psg.tile([128, 3, KC, BL], F32,
                                              tag=f"ps{d}", name=f"ps{d}")
                            ps_r, ps_z, ps_n = (ps_all[:, 0], ps_all[:, 1],
                                                ps_all[:, 2])
                        else:
                            ps_r = psg.tile([128, KC, BL], F32, tag=f"ps_r{d}", name=f"ps_r{d}")
                            ps_z = psg.tile([128, KC, BL], F32, tag=f"ps_z{d}", name=f"ps_z{d}")
                            ps_n = psg.tile([128, KC, BL], F32, tag=f"ps_n{d}", name=f"ps_n{d}")
                        psr.append(ps_r); psz.append(ps_z); psn.append(ps_n)

                        # inject xg_t (r, z) and bhh_n (n) via identity MMs
                        nc.tensor.matmul(ps_r[:], id_sb[:],
                                         xg_sb[d][:, 0:4, tl, :],
                                         start=True, stop=False,
                                         skip_group_check=True)
                        nc.tensor.matmul(ps_n[:], id_sb[:], bhn_sb[d][:],
                                         start=True, stop=False,
                                         skip_group_check=True)
                        nc.tensor.matmul(ps_z[:], id_sb[:],
                                         xg_sb[d][:, 4:8, tl, :],
                                         start=True, stop=False,
                                         skip_group_check=True)

                        # recurrent matmuls; region order r, n, z hides the
                        # sigmoid/tanh chain under the z-region matmuls (and
                        # under the other direction's matmuls when ndir=2)
                        for mbase, ps in ((0, ps_r), (8, ps_n), (4, ps_z)):
                            for mi in range(4):
                                mm = mbase + mi
                                for k in range(KC):
                                    nc.tensor.matmul(
                                        ps[:, mi, :],
                                        whh_sb[d][:, k, 128 * mm:128 * (mm + 1)],
                                        hp[d][:, k, :],
                                        start=False,
                                        stop=(k == KC - 1),
                                        skip_group_check=True,
                                    )

                    for d in D:
                        r_sb = gpool.tile([128, KC, BL], F32, tag=f"r{d}", name=f"r{d}")
                        nc.scalar.activation(r_sb[:], psr[d][:], AF.Sigmoid)
                        rhn = gpool.tile([128, KC, BL], F32, tag=f"rhn{d}", name=f"rhn{d}")
                        nc.vector.tensor_mul(rhn[:], r_sb[:], psn[d][:])
                        pren = gpool.tile([128, KC, BL], F32, tag=f"pren{d}", name=f"pren{d}")
                        nc.vector.tensor_add(pren[:], rhn[:],
                                             xg_sb[d][:, 8:12, tl, :])
                        # sig_z emitted before tanh: ACT is strict FIFO, and
                        # z's psum is ready while tanh still waits on pren
                        z_sb = gpool.tile([128, KC, BL], F32, tag=f"z{d}", name=f"z{d}")
                        nc.scalar.activation(z_sb[:], psz[d][:], AF.Sigmoid)
                        n_sb = gpool.tile([128, KC, BL], F32, tag=f"n{d}", name=f"n{d}")
                        nc.scalar.activation(n_sb[:], pren[:], AF.Tanh)
                        d_sb = gpool.tile([128, KC, BL], F32, tag=f"d{d}", name=f"d{d}")
                        nc.vector.tensor_sub(d_sb[:], hp[d], n_sb[:])
                        zd = gpool.tile([128, KC, BL], F32, tag=f"zd{d}", name=f"zd{d}")
                        nc.vector.tensor_mul(zd[:], z_sb[:], d_sb[:])
                        nc.vector.tensor_add(hist[d][:, :, tl, :], n_sb[:],
                                             zd[:])

                for d in D:
                    nc.sync.dma_start(
                        out=out_h[d][:, :, blk * TBLK:(blk + 1) * TBLK, :],
                        in_=hist[d][:])
                    prev[d] = hist[d]
    nc.compile()
    return nc


def _get_nc(use_gather: bool, ndir: int = 2, reps: int = 1):
    key = ("nc", use_gather, ndir, reps)
    if key not in _CACHE:
        _CACHE[key] = _build_nc(use_gather, ndir, reps)
    return _CACHE[key]


def _prep_dir(Wih, Whh, bih, bhh, BL):
    # lhsT tiles: [p, k, g] = W^T[k*128+p, g] = W[g, k*128+p]
    wih_t = np.ascontiguousarray(
        Wih.T.reshape(KC, 128, 3 * H).transpose(1, 0, 2)
    ).astype(np.float16)
    whh_t = np.ascontiguousarray(
        Whh.T.reshape(KC, 128, 3 * H).transpose(1, 0, 2)
    ).astype(np.float16)
    bias = (bih + bhh).astype(np.float32).copy()
    bias[2 * H:] = bih[2 * H:]  # n chunk: bih only (bhh_n enters before r-mult)
    xgbias = np.ascontiguousarray(bias.reshape(M3H, 128).T).astype(np.float32)
    bhhn = bhh[2 * H:].reshape(KC, 128).T  # [p, c]
    bhhn_bc = np.ascontiguousarray(
        np.broadcast_to(bhhn[:, :, None], (128, KC, BL))
    ).astype(np.float16)
    return wih_t, whh_t, xgbias, bhhn_bc


def _idx_arr(toks):
    n = len(toks)
    ii = np.arange(n)
    a = np.zeros((16, n // 16), np.int16)
    a[ii % 16, ii // 16] = toks
    return np.tile(a, (8, 1))  # replicated across the 8 Q7 cores


def kernel(src, len_src, embed_w, Wih_f, Whh_f, bih_f, bhh_f,
           Wih_b, Whh_b, bih_b, bhh_b):
    global last_exec_time_ns
    src = np.asarray(src)
    len_src = np.asarray(len_src)
    embed_w = np.asarray(embed_w, dtype=np.float32)

    # per-sample reversal of the first len tokens (index prep, host-side)
    t = np.arange(T)[None, :]
    L = len_src[:, None].astype(np.int64)
    idx = np.where(t < L, L - 1 - t, t)  # [B, T]
    src_rev = np.take_along_axis(src.T, idx, axis=1).T  # [T, B]

    use_gather = os.environ.get("KERNEL_NO_GATHER", "0") != "1"
    ndir = int(os.environ.get("KERNEL_NDIR", "1"))
    reps = int(os.environ.get("KERNEL_REPS", "1"))
    trace = os.environ.get("KERNEL_TRACE", "0") == "1"
    BL = 2 * B // (NCORES * ndir)
    NIDX = T * BL

    etab16 = embed_w.astype(np.float16)
    dirs = [
        _prep_dir(np.asarray(Wih_f), np.asarray(Whh_f),
                  np.asarray(bih_f), np.asarray(bhh_f), BL),
        _prep_dir(np.asarray(Wih_b), np.asarray(Whh_b),
                  np.asarray(bih_b), np.asarray(bhh_b), BL),
    ]
    srcs = [src, src_rev]
    ident = np.eye(128, dtype=np.float16)

    in_maps = []
    for c in range(NCORES):
        if ndir == 2:
            core_dirs = [(0, c * BL), (1, c * BL)]
        else:
            d = 0 if c < 4 else 1
            core_dirs = [(d, (c % 4) * BL)]
        m = {"etab": etab16, "ident": ident}
        for key, i in (("wih_t", 0), ("whh_t", 1), ("xgbias", 2), ("bhhn", 3)):
            m[key] = np.stack([dirs[d][i] for d, _ in core_dirs])
        ia, ea = [], []
        for d, b0 in core_dirs:
            toks = np.ascontiguousarray(
                srcs[d][:, b0:b0 + BL]).reshape(-1).astype(np.int16)
            ia.append(_idx_arr(toks))
            if not use_gather:
                emb = etab16[toks]  # [NIDX, E]
                ea.append(np.ascontiguousarray(
                    emb.reshape(NIDX, KC, 128).transpose(2, 1, 0)))
        m["idxs"] = np.stack(ia)
        if not use_gather:
            m["embt"] = np.stack(ea)
        in_maps.append(m)

    nc = _get_nc(use_gather, ndir, reps)
    res = run_bass_kernel_spmd(nc, in_maps, list(range(NCORES)), trace=trace)
    last_exec_time_ns = res.exec_time_ns

    outputs = np.empty((T, B, 2 * H), np.float32)
    for c in range(NCORES):
        oh = res.results[c]["out_h"]  # [ndir, 128, KC, T, BL] f16
        if ndir == 2:
            core_dirs = [(0, c * BL), (1, c * BL)]
        else:
            d0 = 0 if c < 4 else 1
            core_dirs = [(d0, (c % 4) * BL)]
        for i, (d, b0) in enumerate(core_dirs):
            h = oh[i].transpose(2, 3, 1, 0).reshape(T, BL, H).astype(np.float32)
            outputs[:, b0:b0 + BL, d * H:(d + 1) * H] = h

    hidden = outputs[len_src - 1, np.arange(B), H:2 * H][None]  # [1, B, H]
    return outputs, hidden


# revision 36
# speedup vs baseline: 3269.3582x; 1.1810x over previous
"""Bidirectional GRU encoder (T=256, B=64, E=H=512) for 8 Trainium2 NeuronCores.

Sharding (NDIR=2, default): every core runs BOTH directions over a batch slice
of 8 (core c: forward batch [8c, 8c+8) and backward over host-reversed token
order, same slice). The two recurrences are independent, so each direction's
serial gate chain (sigmoid/tanh on ACT, elementwise on DVE) overlaps the other
direction's matmuls — no engine sits idle waiting on the single serial chain.
NDIR=1 fallback: cores 0-3 forward (batch 16), cores 4-7 backward.

Per-core layout is fully transposed: h and all gate tensors live as
[128 partitions = H-chunk, batch free] so elementwise ops use all 128 lanes.
The recurrent matmul streams Whh^T fp16 tiles as stationary weights (FWL) with
h as the tiny moving operand; xg(t) = Wih @ emb(t) is precomputed per 32-step
block at N=256/512 matmul efficiency and injected into PSUM with identity
matmuls. Embeddings are gathered+transposed on-device per block with
dma_gather(transpose=True) from an fp16 copy of the table.
"""

import os
import sys

sys.path.insert(0, "/opt/trn_rl_repo")

import numpy as np

import concourse.bacc as bacc
import concourse.mybir as mybir
from concourse.tile import TileContext
from concourse.bass_utils import run_bass_kernel_spmd

T, B, VOCAB, E, H = 256, 64, 32000, 512, 512
NCORES = 8
TBLK = 32            # recurrence block (xg precompute + output DMA granularity)
NBLK = T // TBLK
KC = 4               # contraction chunks of 128 (E = H = 512)
M3H = 12             # 3H / 128 output chunks

F16 = mybir.dt.float16
F32 = mybir.dt.float32
AF = mybir.ActivationFunctionType

_CACHE = {}

# module-level telemetry for test.py
last_exec_time_ns = None


def _build_nc(use_gather: bool, ndir: int, reps: int = 1):
    BL = 2 * B // (NCORES * ndir)   # batch per core per direction (16 or 8)
    NIDX = T * BL                # tokens per core per direction
    NT = TBLK * BL               # tokens per block per direction

    nc = bacc.Bacc()
    etab = nc.declare_dram_parameter("etab", [VOCAB, E], F16, isOutput=False)
    idxs = nc.declare_dram_parameter("idxs", [ndir, 128, NIDX // 16],
                                     mybir.dt.int16, isOutput=False)
    if not use_gather:
        embt = nc.declare_dram_parameter("embt", [ndir, 128, KC, NIDX], F16,
                                         isOutput=False)
    wih = nc.declare_dram_parameter("wih_t", [ndir, 128, KC, 3 * H], F16,
                                    isOutput=False)
    whh = nc.declare_dram_parameter("whh_t", [ndir, 128, KC, 3 * H], F16,
                                    isOutput=False)
    xgb = nc.declare_dram_parameter("xgbias", [ndir, 128, M3H], F32,
                                    isOutput=False)
    bhn = nc.declare_dram_parameter("bhhn", [ndir, 128, KC, BL], F16,
                                    isOutput=False)
    idn = nc.declare_dram_parameter("ident", [128, 128], F16, isOutput=False)
    out_h = nc.declare_dram_parameter("out_h", [ndir, 128, KC, T, BL], F16,
                                      isOutput=True)

    D = range(ndir)
    with TileContext(nc) as tc:
        with (
            tc.tile_pool(name="const", bufs=1) as cpool,
            tc.tile_pool(name="xg", bufs=2) as xgpool,
            tc.tile_pool(name="hist", bufs=2) as hpool,
            tc.tile_pool(name="g", bufs=3) as gpool,
            tc.tile_pool(name="psg", bufs=2, space="PSUM") as psg,
            tc.tile_pool(name="psx", bufs=2, space="PSUM") as psx,
        ):
            whh_sb, wih_sb, xgb_sb, bhn_sb, idx_sb, emb_sb = [], [], [], [], [], []
            for d in D:
                w1 = cpool.tile([128, KC, 3 * H], F16, tag=f"whh{d}", name=f"whh{d}")
                nc.sync.dma_start(out=w1, in_=whh[d])
                whh_sb.append(w1)
                w2 = cpool.tile([128, KC, 3 * H], F16, tag=f"wih{d}", name=f"wih{d}")
                nc.sync.dma_start(out=w2, in_=wih[d])
                wih_sb.append(w2)
                b1 = cpool.tile([128, M3H], F32, tag=f"xgb{d}", name=f"xgb{d}")
                nc.sync.dma_start(out=b1, in_=xgb[d])
                xgb_sb.append(b1)
                b2 = cpool.tile([128, KC, BL], F16, tag=f"bhn{d}", name=f"bhn{d}")
                nc.sync.dma_start(out=b2, in_=bhn[d])
                bhn_sb.append(b2)
                if use_gather:
                    ix = cpool.tile([128, NIDX // 16], mybir.dt.int16,
                                    tag=f"idx{d}", name=f"idx{d}")
                    nc.sync.dma_start(out=ix, in_=idxs[d])
                    idx_sb.append(ix)
                else:
                    es = cpool.tile([128, KC, NIDX], F16, tag=f"emb{d}", name=f"emb{d}")
                    nc.sync.dma_start(out=es, in_=embt[d])
                    emb_sb.append(es)
            id_sb = cpool.tile([128, 128], F16, tag="idn", name="idn")
            nc.sync.dma_start(out=id_sb, in_=idn[:])
            if use_gather:
                from concourse import library_config
                nc.gpsimd.load_library(library_config.mlp)

            z0 = cpool.tile([128, KC, BL], F16, tag="z0", name="z0")
            nc.gpsimd.memset(z0, 0.0)
            # dependency-free warmup activation: absorbs the ACT table-set
            # load (walrus folds it into the first ACT instruction's waits,
            # which otherwise exceeds the ISA wait-slot limit). Reads its own
            # uninitialized scratch tile so it schedules first.
            warm = cpool.tile([128, 1], F32, tag="warm", name="warm")
            nc.scalar.activation(warm[:], warm[:], AF.Sigmoid)
            nc.scalar.activation(warm[:], warm[:], AF.Tanh)
            nc.scalar.activation(warm[:], warm[:], AF.Identity)

            for rep in range(reps):
              prev = [None] * ndir
              for blk in range(NBLK):
                # ---- xg for this block: xg^T[m] = Wih^T-chunk.T @ emb^T ----
                emb_blk, xg_sb, hist = [], [], []
                for d in D:
                    if use_gather:
                        eb = xgpool.tile([128, KC, NT], F16, tag=f"embblk{d}", name=f"embblk{d}")
                        gsem = nc.alloc_semaphore(f"gsem_{rep}_{blk}_{d}")
                        nc.gpsimd.dma_gather(
                            eb[:], etab[:],
                            idx_sb[d][:, blk * (NT // 16):(blk + 1) * (NT // 16)],
                            NT, NT, E, transpose=True, single_packet=False,
                            prepare_only=True, sem=gsem)
                        nc.gpsimd.trigger_dma(count=1)
                        nc.gpsimd.wait_ge(gsem, 16)
                        # identity affine_select AFTER the engine-blocking
                        # wait: gives Tile a post-landing write access on the
                        # tile so PE consumers order against completed data.
                        nc.gpsimd.affine_select(
                            out=eb[:, 0, 0:1], in_=eb[:, 0, 0:1],
                            compare_op=mybir.AluOpType.is_equal, fill=0.0,
                            base=0, pattern=[[0, 1]], channel_multiplier=0)
                        emb_blk.append(eb)
                    else:
                        emb_blk.append(emb_sb[d][:, :, blk * NT:(blk + 1) * NT])
                    xs = xgpool.tile([128, M3H, TBLK, BL], F16, tag=f"xg{d}", name=f"xg{d}")
                    xg_sb.append(xs)
                    for m in range(M3H):
                        pxg = psx.tile([128, TBLK, BL], F32, tag="pxg", name="pxg")
                        for k in range(KC):
                            nc.tensor.matmul(
                                pxg[:],
                                wih_sb[d][:, k, 128 * m:128 * (m + 1)],
                                emb_blk[d][:, k, :],
                                start=(k == 0),
                                stop=(k == KC - 1),
                            )
                        nc.scalar.activation(
                            xs[:, m, :, :], pxg[:], AF.Identity,
                            bias=xgb_sb[d][:, m:m + 1],
                        )
                    hist.append(hpool.tile([128, KC, TBLK, BL], F16,
                                           tag=f"hist{d}", name=f"hist{d}"))

                for tl in range(TBLK):
                    hp, psr, psz, psn = [], [], [], []
                    for d in D:
                        if blk == 0 and tl == 0:
                            hp.append(z0[:])
                        elif tl == 0:
                            hp.append(prev[d][:, :, TBLK - 1, :])
                        else:
                            hp.append(hist[d][:, :, tl - 1, :])
                        if ndir == 2:
                            ps_all = psg.tile([128, 3, KC, BL], F32,
                                              tag=f"ps{d}", name=f"ps{d}")
                            ps_r, ps_z, ps_n = (ps_all[:, 0], ps_all[:, 1],
                                                ps_all[:, 2])
                        else:
                            ps_r = psg.tile([128, KC, BL], F32, tag=f"ps_r{d}", name=f"ps_r{d}")
                            ps_z = psg.tile([128, KC, BL], F32, tag=f"ps_z{d}", name=f"ps_z{d}")
                            ps_n = psg.tile([128, KC, BL], F32, tag=f"ps_n{d}", name=f"ps_n{d}")
                        psr.append(ps_r); psz.append(ps_z); psn.append(ps_n)

                        # inject xg_t (r, z) and bhh_n (n) via identity MMs
                        nc.tensor.matmul(ps_r[:], id_sb[:],
                                         xg_sb[d][:, 0:4, tl, :],
                                         start=True, stop=False,
                                         skip_group_check=True)
                        nc.tensor.matmul(ps_n[:], id_sb[:], bhn_sb[d][:],
                                         start=True, stop=False,
                                         skip_group_check=True)
                        nc.tensor.matmul(ps_z[:], id_sb[:],
                                         xg_sb[d][:, 4:8, tl, :],
                                         start=True, stop=False,
                                         skip_group_check=True)

                        # recurrent matmuls; region order r, n, z hides the
                        # sigmoid/tanh chain under the z-region matmuls
                        for mbase, ps in ((0, ps_r), (8, ps_n), (4, ps_z)):
                            for mi in range(4):
                                mm = mbase + mi
                                for k in range(KC):
                                    nc.tensor.matmul(
                                        ps[:, mi, :],
                                        whh_sb[d][:, k, 128 * mm:128 * (mm + 1)],
                                        hp[d][:, k, :],
                                        start=False,
                                        stop=(k == KC - 1),
                                        skip_group_check=True,
                                    )

                    for d in D:
                        r_sb = gpool.tile([128, KC, BL], F16, tag=f"r{d}", name=f"r{d}")
                        nc.scalar.activation(r_sb[:], psr[d][:], AF.Sigmoid)
                        rhn = gpool.tile([128, KC, BL], F16, tag=f"rhn{d}", name=f"rhn{d}")
                        nc.vector.tensor_mul(rhn[:], r_sb[:], psn[d][:])
                        pren = gpool.tile([128, KC, BL], F16, tag=f"pren{d}", name=f"pren{d}")
                        nc.vector.tensor_add(pren[:], rhn[:],
                                             xg_sb[d][:, 8:12, tl, :])
                        # z and 1-z both from ACT (scale=-1 trick), emitted
                        # before tanh: ACT is strict FIFO and z's psum is
                        # ready while tanh still waits on pren
                        z_sb = gpool.tile([128, KC, BL], F16, tag=f"z{d}", name=f"z{d}")
                        nc.scalar.activation(z_sb[:], psz[d][:], AF.Sigmoid)
                        oz_sb = gpool.tile([128, KC, BL], F16, tag=f"oz{d}", name=f"oz{d}")
                        nc.scalar.activation(oz_sb[:], psz[d][:], AF.Sigmoid,
                                             scale=-1.0)
                        # c = z*h_prev runs on DVE during the tanh window
                        c_sb = gpool.tile([128, KC, BL], F16, tag=f"c{d}", name=f"c{d}")
                        nc.vector.tensor_mul(c_sb[:], z_sb[:], hp[d])
                        n_sb = gpool.tile([128, KC, BL], F16, tag=f"n{d}", name=f"n{d}")
                        nc.scalar.activation(n_sb[:], pren[:], AF.Tanh)
                        # h = (1-z)*n + z*h_prev: only two post-tanh DVE ops
                        w_sb = gpool.tile([128, KC, BL], F16, tag=f"w{d}", name=f"w{d}")
                        nc.vector.tensor_mul(w_sb[:], oz_sb[:], n_sb[:])
                        nc.vector.tensor_add(hist[d][:, :, tl, :], w_sb[:],
                                             c_sb[:])

                for d in D:
                    nc.sync.dma_start(
                        out=out_h[d][:, :, blk * TBLK:(blk + 1) * TBLK, :],
                        in_=hist[d][:])
                    prev[d] = hist[d]
    nc.compile()
    return nc


def _get_nc(use_gather: bool, ndir: int = 2, reps: int = 1):
    key = ("nc", use_gather, ndir, reps)
    if key not in _CACHE:
        _CACHE[key] = _build_nc(use_gather, ndir, reps)
    return _CACHE[key]


def _prep_dir(Wih, Whh, bih, bhh, BL):
    # lhsT tiles: [p, k, g] = W^T[k*128+p, g] = W[g, k*128+p]
    wih_t = np.ascontiguousarray(
        Wih.T.reshape(KC, 128, 3 * H).transpose(1, 0, 2)
    ).astype(np.float16)
    whh_t = np.ascontiguousarray(
        Whh.T.reshape(KC, 128, 3 * H).transpose(1, 0, 2)
    ).astype(np.float16)
    bias = (bih + bhh).astype(np.float32).copy()
    bias[2 * H:] = bih[2 * H:]  # n chunk: bih only (bhh_n enters before r-mult)
    xgbias = np.ascontiguousarray(bias.reshape(M3H, 128).T).astype(np.float32)
    bhhn = bhh[2 * H:].reshape(KC, 128).T  # [p, c]
    bhhn_bc = np.ascontiguousarray(
        np.broadcast_to(bhhn[:, :, None], (128, KC, BL))
    ).astype(np.float16)
    return wih_t, whh_t, xgbias, bhhn_bc


def _idx_arr(toks):
    n = len(toks)
    ii = np.arange(n)
    a = np.zeros((16, n // 16), np.int16)
    a[ii % 16, ii // 16] = toks
    return np.tile(a, (8, 1))  # replicated across the 8 Q7 cores


def kernel(src, len_src, embed_w, Wih_f, Whh_f, bih_f, bhh_f,
           Wih_b, Whh_b, bih_b, bhh_b):
    global last_exec_time_ns
    src = np.asarray(src)
    len_src = np.asarray(len_src)
    embed_w = np.asarray(embed_w, dtype=np.float32)

    # per-sample reversal of the first len tokens (index prep, host-side)
    t = np.arange(T)[None, :]
    L = len_src[:, None].astype(np.int64)
    idx = np.where(t < L, L - 1 - t, t)  # [B, T]
    src_rev = np.take_along_axis(src.T, idx, axis=1).T  # [T, B]

    use_gather = os.environ.get("KERNEL_NO_GATHER", "0") != "1"
    ndir = int(os.environ.get("KERNEL_NDIR", "1"))
    reps = int(os.environ.get("KERNEL_REPS", "1"))
    trace = os.environ.get("KERNEL_TRACE", "0") == "1"
    BL = 2 * B // (NCORES * ndir)
    NIDX = T * BL

    etab16 = embed_w.astype(np.float16)
    dirs = [
        _prep_dir(np.asarray(Wih_f), np.asarray(Whh_f),
                  np.asarray(bih_f), np.asarray(bhh_f), BL),
        _prep_dir(np.asarray(Wih_b), np.asarray(Whh_b),
                  np.asarray(bih_b), np.asarray(bhh_b), BL),
    ]
    srcs = [src, src_rev]
    ident = np.eye(128, dtype=np.float16)

    in_maps = []
    for c in range(NCORES):
        if ndir == 2:
            core_dirs = [(0, c * BL), (1, c * BL)]
        else:
            d = 0 if c < 4 else 1
            core_dirs = [(d, (c % 4) * BL)]
        m = {"etab": etab16, "ident": ident}
        for key, i in (("wih_t", 0), ("whh_t", 1), ("xgbias", 2), ("bhhn", 3)):
            m[key] = np.stack([dirs[d][i] for d, _ in core_dirs])
        ia, ea = [], []
        for d, b0 in core_dirs:
            toks = np.ascontiguousarray(
                srcs[d][:, b0:b0 + BL]).reshape(-1).astype(np.int16)
            ia.append(_idx_arr(toks))
            if not use_gather:
                emb = etab16[toks]  # [NIDX, E]
                ea.append(np.ascontiguousarray(
                    emb.reshape(NIDX, KC, 128).transpose(2, 1, 0)))
        m["idxs"] = np.stack(ia)
        if not use_gather:
            m["embt"] = np.stack(ea)
        in_maps.append(m)

    nc = _get_nc(use_gather, ndir, reps)
    res = run_bass_kernel_spmd(nc, in_maps, list(range(NCORES)), trace=trace)
    last_exec_time_ns = res.exec_time_ns

    outputs = np.empty((T, B, 2 * H), np.float32)
    for c in range(NCORES):
        oh = res.results[c]["out_h"]  # [ndir, 128, KC, T, BL] f16
        if ndir == 2:
            core_dirs = [(0, c * BL), (1, c * BL)]
        else:
            d0 = 0 if c < 4 else 1
            core_dirs = [(d0, (c % 4) * BL)]
        for i, (d, b0) in enumerate(core_dirs):
            h = oh[i].transpose(2, 3, 1, 0).reshape(T, BL, H).astype(np.float32)
            outputs[:, b0:b0 + BL, d * H:(d + 1) * H] = h

    hidden = outputs[len_src - 1, np.arange(B), H:2 * H][None]  # [1, B, H]
    return outputs, hidden
